# revision 13
# baseline (speedup 1.0000x reference)
"""DequantSiluAndMulQuant Trainium2 Bass kernel.

Full input x: [8192, 22016] fp32 (gate | up halves, int8-GEMM accumulator
scale). Output: (q int8 [8192, 11008], scale fp32 [8192]) — per-token
symmetric int8 quant of silu(gate*gs) * (up*us).

Sharding: rows are data-parallel; 8 NeuronCores × 1024 rows each, no
communication. Per core: 8 blocks of 128 tokens (tokens on SBUF
partitions). Per block, the D=11008 columns stream through in chunks:
DMA gate+up, ScalarE silu (dequant scale folded in) written straight
into the persistent fp32 act buffer, VectorE in-place multiply by up,
then a per-chunk abs-max reduce so the per-token amax is ready the
moment the last chunk lands. Quantize is a single ScalarE copy per chunk
with per-partition scale (fp32->int8 conversion on HW is round-half-even
with saturation, matching round+clip in the reference), DMA'd out
chunk-by-chunk.

The up-half dequant scale cancels inside q (q = round(act*127/amax) is
invariant to uniform scaling), so `up` is used raw and `us` only enters
the emitted scale = amax*us/127.
"""

import numpy as np

N_TOKENS = 8192
D = 11008
N_CORES = 8
ROWS = N_TOKENS // N_CORES  # 1024
P = 128
NBLK = ROWS // P  # 8
NCH = 4
C = D // NCH  # 2752

_cache = {}


def _build(gate_scale: float, up_scale: float, loop_n: int | None = None):
    key = (gate_scale, up_scale, loop_n)
    if key in _cache:
        return _cache[key]

    import concourse.bacc as bacc
    import concourse.mybir as mybir
    from concourse import tile

    nc = bacc.Bacc("TRN2", target_bir_lowering=False, debug=False,
                   num_devices=N_CORES)

    x = nc.dram_tensor("x", [ROWS, 2 * D], mybir.dt.float32,
                       kind="ExternalInput").ap()
    q = nc.dram_tensor("q", [ROWS, D], mybir.dt.int8,
                       kind="ExternalOutput").ap()
    scale = nc.dram_tensor("scale", [ROWS], mybir.dt.float32,
                           kind="ExternalOutput").ap()

    f32 = mybir.dt.float32
    i8 = mybir.dt.int8
    Alu = mybir.AluOpType
    Act = mybir.ActivationFunctionType
    X = mybir.AxisListType.X

    from contextlib import nullcontext

    with tile.TileContext(nc) as tc:
        with (
            tc.tile_pool(name="inp", bufs=3) as inp,
            tc.tile_pool(name="actp", bufs=2) as actp,
            tc.tile_pool(name="qp", bufs=14) as qp,
            tc.tile_pool(name="small", bufs=2 * NBLK) as small,
            tc.For_i(0, loop_n, 1) if loop_n else nullcontext(),
        ):
            # The SP HWDGE queue is in-order, so DMA issue order is
            # scheduling. q-chunk DMAs for early blocks are issued after the
            # NEXT block's chunk-2 loads (they are ready by then and never
            # head-block input traffic); blocks 5 and 6 are held back until
            # all inputs are issued so the very last input chunk lands ~10us
            # sooner and block 7's quantize latency hides behind their
            # queued output traffic.
            pending = {}

            def flush(b):
                for dma_args in pending.pop(b, []):
                    nc.sync.dma_start(**dma_args)

            for b in range(NBLK):
                r0 = b * P
                last = b == NBLK - 1
                act = actp.tile([P, D], f32, tag="act")
                parts = small.tile([P, NCH + 1], f32, tag="parts")
                # the final block's last chunk is small so the post-arrival
                # silu/mul/reduce chain at the very end of the kernel is
                # short
                if last:
                    cuts = [0, C, 2 * C, 3 * C, 3 * C + 2408, D]
                else:
                    cuts = [0, C, 2 * C, 3 * C, D]
                for k in range(len(cuts) - 1):
                    c0, c1 = cuts[k], cuts[k + 1]
                    w = c1 - c0
                    g = inp.tile([P, w], f32, tag="g")
                    u = inp.tile([P, w], f32, tag="u")
                    nc.sync.dma_start(out=g[:, :w],
                                      in_=x[r0:r0 + P, c0:c1])
                    nc.sync.dma_start(out=u[:, :w],
                                      in_=x[r0:r0 + P, D + c0:D + c1])
                    if k == 2 and 1 <= b <= 4:
                        flush(b - 1)
                    ac = act[:, c0:c1]
                    nc.scalar.activation(ac, g[:, :w], Act.Silu,
                                         scale=gate_scale)
                    nc.vector.tensor_tensor(ac, ac, u[:, :w], Alu.mult)
                    nc.vector.tensor_reduce(parts[:, k:k + 1], ac, axis=X,
                                            op=Alu.max,
                                            apply_absolute_value=True)
                if last:
                    flush(NBLK - 4)
                    flush(NBLK - 3)
                    flush(NBLK - 2)
                amax = small.tile([P, 1], f32, tag="amax")
                nc.vector.tensor_reduce(amax[:], parts[:, :len(cuts) - 1],
                                        axis=X, op=Alu.max)
                r = small.tile([P, 1], f32, tag="r")
                qm = small.tile([P, 1], f32, tag="qm")
                so = small.tile([P, 1], f32, tag="so")
                nc.vector.reciprocal(r[:], amax[:])
                nc.vector.tensor_scalar(qm[:], r[:], 127.0, None, Alu.mult)
                nc.vector.tensor_scalar(so[:], amax[:], up_scale / 127.0,
                                        None, Alu.mult)
                out_dmas = [dict(out=scale[r0:r0 + P], in_=so[:])]
                if last:
                    # balanced two-engine quantize split for the kernel tail:
                    # ScalarE (1.2 GHz) takes a bit more than half, VectorE
                    # picks up the rest as soon as its reduce chain drains
                    qcuts = [(0, 2752, "act"), (5504, 7912, "dve"),
                             (2752, 5504, "act"), (7912, 10320, "dve"),
                             (10320, D, "act")]
                else:
                    qcuts = [(k * C, (k + 1) * C, "act") for k in range(NCH)]
                for c0, c1, eng in qcuts:
                    w = c1 - c0
                    qt = qp.tile([P, C], i8, tag="q")
                    if eng == "dve":
                        nc.vector.tensor_scalar(qt[:, :w], act[:, c0:c1],
                                                qm[:], None, Alu.mult)
                    else:
                        nc.scalar.activation(qt[:, :w], act[:, c0:c1],
                                             Act.Copy, scale=qm[:])
                    out_dmas.append(dict(out=q[r0:r0 + P, c0:c1],
                                         in_=qt[:, :w]))
                if last:
                    for dma_args in out_dmas:
                        nc.sync.dma_start(**dma_args)
                else:
                    pending[b] = out_dmas

    nc.compile()
    _cache[key] = nc
    return nc


def kernel(x, gate_dequant_scale, up_dequant_scale):
    from concourse.bass_utils import run_bass_kernel_spmd

    x = np.asarray(x, dtype=np.float32)
    gs = float(np.asarray(gate_dequant_scale))
    us = float(np.asarray(up_dequant_scale))
    nc = _build(gs, us)

    in_maps = [{"x": x[c * ROWS:(c + 1) * ROWS]} for c in range(N_CORES)]
    res = run_bass_kernel_spmd(nc, in_maps, core_ids=list(range(N_CORES)))
    q = np.concatenate([res.results[c]["q"] for c in range(N_CORES)], axis=0)
    scale = np.concatenate([res.results[c]["scale"].reshape(-1)
                            for c in range(N_CORES)], axis=0)
    return q.astype(np.int8), scale.astype(np.float32)


# revision 17
# speedup vs baseline: 1.0262x; 1.0262x over previous
"""DequantSiluAndMulQuant Trainium2 Bass kernel.

Full input x: [8192, 22016] fp32 (gate | up halves, int8-GEMM accumulator
scale). Output: (q int8 [8192, 11008], scale fp32 [8192]) — per-token
symmetric int8 quant of silu(gate*gs) * (up*us).

Sharding: rows are data-parallel; 8 NeuronCores × 1024 rows each, no
communication. Per core: 8 blocks of 128 tokens (tokens on SBUF
partitions). Per block, the D=11008 columns stream through in chunks:
DMA gate+up, ScalarE silu (dequant scale folded in) written straight
into the persistent fp32 act buffer, VectorE in-place multiply by up,
then a per-chunk abs-max reduce so the per-token amax is ready the
moment the last chunk lands. Quantize is a single ScalarE copy per chunk
with per-partition scale (fp32->int8 conversion on HW is round-half-even
with saturation, matching round+clip in the reference), DMA'd out
chunk-by-chunk.

The up-half dequant scale cancels inside q (q = round(act*127/amax) is
invariant to uniform scaling), so `up` is used raw and `us` only enters
the emitted scale = amax*us/127.
"""

import numpy as np

N_TOKENS = 8192
D = 11008
N_CORES = 8
ROWS = N_TOKENS // N_CORES  # 1024
P = 128
NBLK = ROWS // P  # 8
NCH = 4
C = D // NCH  # 2752

_cache = {}


def _build(gate_scale: float, up_scale: float, loop_n: int | None = None):
    key = (gate_scale, up_scale, loop_n)
    if key in _cache:
        return _cache[key]

    import concourse.bacc as bacc
    import concourse.mybir as mybir
    from concourse import tile

    nc = bacc.Bacc("TRN2", target_bir_lowering=False, debug=False,
                   num_devices=N_CORES)

    x = nc.dram_tensor("x", [ROWS, 2 * D], mybir.dt.float32,
                       kind="ExternalInput").ap()
    q = nc.dram_tensor("q", [ROWS, D], mybir.dt.int8,
                       kind="ExternalOutput").ap()
    scale = nc.dram_tensor("scale", [ROWS], mybir.dt.float32,
                           kind="ExternalOutput").ap()

    f32 = mybir.dt.float32
    i8 = mybir.dt.int8
    Alu = mybir.AluOpType
    Act = mybir.ActivationFunctionType
    X = mybir.AxisListType.X

    from contextlib import nullcontext

    with tile.TileContext(nc) as tc:
        with (
            tc.tile_pool(name="inp", bufs=3) as inp,
            tc.tile_pool(name="actp", bufs=2) as actp,
            tc.tile_pool(name="qp", bufs=14) as qp,
            tc.tile_pool(name="small", bufs=2 * NBLK) as small,
            tc.For_i(0, loop_n, 1) if loop_n else nullcontext(),
        ):
            # Real-HW per-HWDGE-queue throughput (~150-160 GB/s) is about
            # half of what this kernel needs, so input traffic is split
            # across both queues: gate loads issue from SP, up loads from
            # ACT; q-chunk outputs alternate between the two. Queues are
            # in-order, so DMA issue order is scheduling: q DMAs for early
            # blocks are issued after the NEXT block's chunk-2 loads (ready
            # by then, never head-block input traffic); blocks 4-6 are held
            # back until all inputs are issued so the last input chunk lands
            # sooner and block 7's quantize latency hides behind their
            # queued output traffic.
            pending = {}

            def flush(b):
                for eng, dma_args in pending.pop(b, []):
                    eng.dma_start(**dma_args)

            for b in range(NBLK):
                r0 = b * P
                last = b == NBLK - 1
                act = actp.tile([P, D], f32, tag="act")
                parts = small.tile([P, NCH + 1], f32, tag="parts")
                # the final block's last chunk is small so the post-arrival
                # silu/mul/reduce chain at the very end of the kernel is
                # short
                if last:
                    cuts = [0, C, 2 * C, 3 * C, 3 * C + 2408, D]
                else:
                    cuts = [0, C, 2 * C, 3 * C, D]
                for k in range(len(cuts) - 1):
                    c0, c1 = cuts[k], cuts[k + 1]
                    w = c1 - c0
                    g = inp.tile([P, w], f32, tag="g")
                    u = inp.tile([P, w], f32, tag="u")
                    nc.sync.dma_start(out=g[:, :w],
                                      in_=x[r0:r0 + P, c0:c1])
                    nc.scalar.dma_start(out=u[:, :w],
                                        in_=x[r0:r0 + P, D + c0:D + c1])
                    if k == 2 and 1 <= b <= 4:
                        flush(b - 1)
                    ac = act[:, c0:c1]
                    nc.scalar.activation(ac, g[:, :w], Act.Silu,
                                         scale=gate_scale)
                    nc.vector.tensor_tensor(ac, ac, u[:, :w], Alu.mult)
                    nc.vector.tensor_reduce(parts[:, k:k + 1], ac, axis=X,
                                            op=Alu.max,
                                            apply_absolute_value=True)
                if last:
                    flush(NBLK - 4)
                    flush(NBLK - 3)
                    flush(NBLK - 2)
                amax = small.tile([P, 1], f32, tag="amax")
                nc.vector.tensor_reduce(amax[:], parts[:, :len(cuts) - 1],
                                        axis=X, op=Alu.max)
                r = small.tile([P, 1], f32, tag="r")
                qm = small.tile([P, 1], f32, tag="qm")
                so = small.tile([P, 1], f32, tag="so")
                nc.vector.reciprocal(r[:], amax[:])
                nc.vector.tensor_scalar(qm[:], r[:], 127.0, None, Alu.mult)
                nc.vector.tensor_scalar(so[:], amax[:], up_scale / 127.0,
                                        None, Alu.mult)
                out_dmas = [(nc.sync, dict(out=scale[r0:r0 + P],
                                           in_=so[:]))]
                if last:
                    # balanced two-engine quantize split for the kernel tail:
                    # ScalarE (1.2 GHz) takes a bit more than half, VectorE
                    # picks up the rest as soon as its reduce chain drains
                    qcuts = [(0, 2752, "act"), (5504, 7912, "dve"),
                             (2752, 5504, "act"), (7912, 10320, "dve"),
                             (10320, D, "act")]
                else:
                    qcuts = [(k * C, (k + 1) * C, "act") for k in range(NCH)]
                for j, (c0, c1, eng) in enumerate(qcuts):
                    w = c1 - c0
                    qt = qp.tile([P, C], i8, tag="q")
                    if eng == "dve":
                        nc.vector.tensor_scalar(qt[:, :w], act[:, c0:c1],
                                                qm[:], None, Alu.mult)
                    else:
                        nc.scalar.activation(qt[:, :w], act[:, c0:c1],
                                             Act.Copy, scale=qm[:])
                    qeng = nc.sync if j % 2 == 0 else nc.scalar
                    out_dmas.append((qeng, dict(out=q[r0:r0 + P, c0:c1],
                                                in_=qt[:, :w])))
                if last:
                    for eng, dma_args in out_dmas:
                        eng.dma_start(**dma_args)
                else:
                    pending[b] = out_dmas

    nc.compile()
    _cache[key] = nc
    return nc


def kernel(x, gate_dequant_scale, up_dequant_scale):
    from concourse.bass_utils import run_bass_kernel_spmd

    x = np.asarray(x, dtype=np.float32)
    gs = float(np.asarray(gate_dequant_scale))
    us = float(np.asarray(up_dequant_scale))
    nc = _build(gs, us)

    in_maps = [{"x": x[c * ROWS:(c + 1) * ROWS]} for c in range(N_CORES)]
    res = run_bass_kernel_spmd(nc, in_maps, core_ids=list(range(N_CORES)))
    q = np.concatenate([res.results[c]["q"] for c in range(N_CORES)], axis=0)
    scale = np.concatenate([res.results[c]["scale"].reshape(-1)
                            for c in range(N_CORES)], axis=0)
    return q.astype(np.int8), scale.astype(np.float32)
